# revision 38
# baseline (speedup 1.0000x reference)
"""Mamba block (MockMambaBlock) on 8 Trainium2 NeuronCores.

Sharding: tensor-parallel over d_inner (8 x 256 channels), both batches on
every core. The x_proj/dt_proj contraction over d_inner is completed with an
on-device AllReduce (chunked per 512 tokens so it overlaps phase A); out_proj
row-partials are summed on the host (the gather step).

v7 design:
  - PE: in_proj, depthwise conv (diag-matmul), x_proj, dt_proj, y n-sum
    (identity matmul), D-term (diag matmul), out_proj.
  - Scalar: silu/sigmoid/ln/exp activations, PSUM->SBUF copies.
  - DVE (vector): the 64 full-length SSM scans (f32), u = dtx*B (bf16 2x),
    dtx, gating. Nothing else competes for DVE's shared port: gpsimd only
    runs collectives (gpsimd tensor ops would lock DVE's 2nd SBUF port).
  - Phase A emits the x-branch first (in_proj x-half, conv, x_proj, AR, dt)
    so md is ready ~75us in; the z-branch (only needed at gating) and all
    of batch 1's phase A are deferred into batch 0's scan stream.
  - dt (Sigmoid/Ln) is emitted one chunk late so it never head-of-line
    blocks the scalar queue while its AllReduce is in flight.
"""

import sys

sys.path.insert(0, "/opt/trn_rl_repo")

import numpy as np
import ml_dtypes

import concourse.bass as bass
import concourse.bacc as bacc
import concourse.mybir as mybir
import concourse.tile as tile
from concourse.bass_utils import run_bass_kernel_spmd

F32 = mybir.dt.float32
BF16 = mybir.dt.bfloat16
AF = mybir.ActivationFunctionType
OP = mybir.AluOpType

B, L, DM, DI, DS, DC = 2, 2048, 1024, 2048, 16, 4
NCORES = 8
DIL = DI // NCORES          # 256 channels per core
NBLK = DIL // 128           # 2 partition blocks of channels
KBLK = DM // 128            # 8 contraction blocks for in_proj
LTA = 512                   # phase A token chunk
NCHA = L // LTA
NPT = L // 512


def build_nc():
    nc = bacc.Bacc()

    x_t = nc.dram_tensor("x_t", [B, KBLK, 128, L], BF16, kind="ExternalInput")
    win_d = nc.dram_tensor("win", [DM, 2 * DIL], BF16, kind="ExternalInput")
    wout_d = nc.dram_tensor("wout", [DIL, DM], BF16, kind="ExternalInput")
    wx_d = nc.dram_tensor("wx", [DIL, 2 * DS], BF16, kind="ExternalInput")
    wdt_d = nc.dram_tensor("wdt", [DS, DIL], BF16, kind="ExternalInput")
    a_d = nc.dram_tensor("a", [DIL, DS], F32, kind="ExternalInput")
    convdiag_d = nc.dram_tensor("convdiag", [DIL, DC * 128], BF16,
                                kind="ExternalInput")
    convb_d = nc.dram_tensor("convb", [DIL, 1], F32, kind="ExternalInput")
    bdt_d = nc.dram_tensor("bdt", [DIL, 1], F32, kind="ExternalInput")
    identb_d = nc.dram_tensor("identb", [128, 128], BF16, kind="ExternalInput")
    diagd_d = nc.dram_tensor("diagd", [DIL, 128], BF16, kind="ExternalInput")
    F16 = mybir.dt.float16
    out_d = nc.dram_tensor("out_p", [B, L, DM], F16, kind="ExternalOutput")

    with tile.TileContext(nc) as tc:
        with (
            tc.tile_pool(name="weights", bufs=1) as wp,
            tc.tile_pool(name="resident", bufs=1) as rp,
            tc.tile_pool(name="dram", bufs=1, space="DRAM") as dp,
            tc.tile_pool(name="pa", bufs=2) as pa,
            tc.tile_pool(name="pb", bufs=2) as pb,
            tc.tile_pool(name="ps_in", bufs=2, space="PSUM") as ps_in,
            tc.tile_pool(name="ps_cv", bufs=1, space="PSUM") as ps_cv,
            tc.tile_pool(name="ps_small", bufs=1, space="PSUM") as ps_small,
            tc.tile_pool(name="ps_y", bufs=1, space="PSUM") as ps_y,
        ):
            # ---- weights to SBUF (x-path critical ones first; the
            # phase-B-only ones are DMA'd behind the first xs loads) ----
            win_sb = wp.tile([128, KBLK, 2 * DIL], BF16)
            nc.sync.dma_start(win_sb[:], win_d[:].rearrange("(k p) m -> p k m", p=128))
            convdiag_sb = wp.tile([128, NBLK, DC * 128], BF16)
            nc.sync.dma_start(convdiag_sb[:],
                              convdiag_d[:].rearrange("(k p) m -> p k m", p=128))
            convb_sb = wp.tile([128, NBLK, 1], F32)
            nc.sync.dma_start(convb_sb[:], convb_d[:].rearrange("(k p) m -> p k m", p=128))
            wx_sb = wp.tile([128, NBLK, 2 * DS], BF16)
            nc.sync.dma_start(wx_sb[:], wx_d[:].rearrange("(k p) m -> p k m", p=128))

            def load_late_weights():
                wdt_sb = wp.tile([DS, DIL], BF16)
                nc.sync.dma_start(wdt_sb[:], wdt_d[:])
                a_sb = wp.tile([128, NBLK, DS], F32)
                nc.sync.dma_start(a_sb[:], a_d[:].rearrange("(k p) m -> p k m", p=128))
                bdt_sb = wp.tile([128, NBLK, 1], F32)
                nc.sync.dma_start(bdt_sb[:], bdt_d[:].rearrange("(k p) m -> p k m", p=128))
                identb_sb = wp.tile([128, 128], BF16)
                nc.sync.dma_start(identb_sb[:], identb_d[:])
                diagd_sb = wp.tile([128, NBLK, 128], BF16)
                nc.sync.dma_start(diagd_sb[:], diagd_d[:].rearrange("(k p) m -> p k m", p=128))
                wout_sb = wp.tile([128, NBLK, DM], BF16)
                nc.sync.dma_start(wout_sb[:], wout_d[:].rearrange("(k p) m -> p k m", p=128))
                return wdt_sb, a_sb, bdt_sb, identb_sb, diagd_sb, wout_sb

            # ---- resident activations ----
            xcv = [[rp.tile([128, L], BF16, name=f"xcv{b_}{k}", tag=f"xcv{b_}{k}")
                    for k in range(NBLK)] for b_ in range(B)]
            zac = [[rp.tile([128, L], BF16, name=f"zac{b_}{k}", tag=f"zac{b_}{k}")
                    for k in range(NBLK)] for b_ in range(B)]
            # AllReduced x_ssm in DRAM: rows 0:DS = dt_in, rows DS: = B_ssm
            # (DRAM so the bb partition-broadcast DMA can read it)
            ccall = [dp.tile([2 * DS, L], BF16, name=f"ccall{b_}")
                     for b_ in range(B)]
            # dt_in rows staged in SBUF for the dt_proj matmul
            dtin_sb = [rp.tile([DS, L], BF16, name=f"dtin{b_}",
                               tag=f"dtin{b_}") for b_ in range(B)]
            md = [[rp.tile([128, L], BF16, name=f"md{b_}{k}", tag=f"md{b_}{k}")
                   for k in range(NBLK)] for b_ in range(B)]

            warm_in = dp.tile([2, DS], F32, name="warm_in")
            warm_out = dp.tile([2, DS], F32, addr_space="Shared",
                               name="warm_out")
            cc_in = [dp.tile([2 * DS, L], BF16, name=f"cc_in{b_}")
                     for b_ in range(B)]
            cc_out = [dp.tile([2 * DS, L], BF16, addr_space="Shared",
                              name=f"cc_out{b_}") for b_ in range(B)]

            xp_buf = [pa.tile([128, LTA + DC - 1], BF16, name=f"xpb{k}",
                              tag=f"xpb{k}", bufs=1) for k in range(NBLK)]

            def in_proj_m(b_, ch, m, xs_all):
                ps = ps_in.tile([128, LTA], F32, tag="ps_in")
                for kb in range(KBLK):
                    nc.tensor.matmul(
                        ps[:], win_sb[:, kb, m * 128:(m + 1) * 128],
                        xs_all[:, kb, :],
                        start=(kb == 0), stop=(kb == KBLK - 1))
                return ps

            def load_xs(b_, ch, tag="xs"):
                t0 = ch * LTA
                xs_all = pa.tile([128, KBLK, LTA], BF16, tag=tag,
                                 bufs=4 if tag == "xs" else 2,
                                 name=f"{tag}{b_}{ch}")
                nc.sync.dma_start(
                    xs_all[:], x_t[b_].transpose([1, 0, 2])[:, :, t0:t0 + LTA])
                return xs_all

            def emit_a_x(b_, ch, xs_all):
                """x-branch of phase A for one 512-token chunk, with the
                per-stage ops batched across both channel-blocks (fewer
                activation-table switches) and the conv PSUM routed through
                the (still idle) y banks for 4-deep rotation."""
                t0 = ch * LTA
                pss = [in_proj_m(b_, ch, blk, xs_all) for blk in range(NBLK)]
                for blk in range(NBLK):
                    if ch == 0:
                        nc.vector.memset(xp_buf[blk][:, 0:DC - 1], 0.0)
                    else:
                        nc.vector.tensor_copy(
                            xp_buf[blk][:, 0:DC - 1],
                            xp_buf[blk][:, LTA:LTA + DC - 1])
                    nc.scalar.copy(xp_buf[blk][:, DC - 1:LTA + DC - 1],
                                   pss[blk][:])
                pscs = []
                for blk in range(NBLK):
                    psc = ps_y.tile([128, LTA], F32,
                                    tag=f"y{(2 * ch + blk) % 4}",
                                    name=f"psc{b_}{ch}{blk}")
                    for k in range(DC):
                        nc.tensor.matmul(
                            psc[:],
                            convdiag_sb[:, blk, k * 128:(k + 1) * 128],
                            xp_buf[blk][:, k:k + LTA],
                            start=(k == 0), stop=(k == DC - 1))
                    pscs.append(psc)
                for blk in range(NBLK):
                    nc.scalar.activation(
                        xcv[b_][blk][:, t0:t0 + LTA], pscs[blk][:],
                        AF.Silu, bias=convb_sb[:, blk, :])
                # x_proj partial for this chunk
                psx = ps_small.tile([128, LTA], F32, tag="ps_small")
                for kb in range(NBLK):
                    nc.tensor.matmul(
                        psx[0:2 * DS, :], wx_sb[:, kb, :],
                        xcv[b_][kb][:, t0:t0 + LTA],
                        start=(kb == 0), stop=(kb == NBLK - 1))
                xssb = pa.tile([2 * DS, LTA], BF16, tag="xssb", bufs=2)
                nc.scalar.copy(xssb[:], psx[0:2 * DS, :])
                nc.sync.dma_start(cc_in[b_][:, t0:t0 + LTA], xssb[:])

            def emit_a_x_half(b_, ch, blk, xs_all):
                """One channel-block of a chunk's x-branch (for fine-grain
                interleaving into the scan stream)."""
                t0 = ch * LTA
                ps = in_proj_m(b_, ch, blk, xs_all)
                if ch == 0:
                    nc.vector.memset(xp_buf[blk][:, 0:DC - 1], 0.0)
                else:
                    nc.vector.tensor_copy(
                        xp_buf[blk][:, 0:DC - 1],
                        xp_buf[blk][:, LTA:LTA + DC - 1])
                nc.scalar.copy(xp_buf[blk][:, DC - 1:LTA + DC - 1], ps[:])
                psc = ps_cv.tile([128, LTA], F32, tag="ps_cv")
                for k in range(DC):
                    nc.tensor.matmul(
                        psc[:],
                        convdiag_sb[:, blk, k * 128:(k + 1) * 128],
                        xp_buf[blk][:, k:k + LTA],
                        start=(k == 0), stop=(k == DC - 1))
                nc.scalar.activation(
                    xcv[b_][blk][:, t0:t0 + LTA], psc[:],
                    AF.Silu, bias=convb_sb[:, blk, :])
                if blk == NBLK - 1:
                    psx = ps_small.tile([128, LTA], F32, tag="ps_small")
                    for kb in range(NBLK):
                        nc.tensor.matmul(
                            psx[0:2 * DS, :], wx_sb[:, kb, :],
                            xcv[b_][kb][:, t0:t0 + LTA],
                            start=(kb == 0), stop=(kb == NBLK - 1))
                    xssb = pa.tile([2 * DS, LTA], BF16, tag="xssb", bufs=2)
                    nc.scalar.copy(xssb[:], psx[0:2 * DS, :])
                    nc.sync.dma_start(cc_in[b_][:, t0:t0 + LTA], xssb[:])

            def emit_a_z_half(b_, ch, blk, xs_all):
                t0 = ch * LTA
                ps = in_proj_m(b_, ch, NBLK + blk, xs_all)
                nc.scalar.activation(
                    zac[b_][blk][:, t0:t0 + LTA], ps[:], AF.Silu)

            def emit_a_z(b_, ch):
                """z-branch (gate input): in_proj z half + silu. Deferred —
                only needed at gating time; reloads x from DRAM so phase A's
                xs tiles can rotate freely."""
                t0 = ch * LTA
                xs_all = load_xs(b_, ch, tag="xsz")
                for blk in range(NBLK):
                    ps = in_proj_m(b_, ch, NBLK + blk, xs_all)
                    nc.scalar.activation(
                        zac[b_][blk][:, t0:t0 + LTA], ps[:], AF.Silu)

            def emit_a_comm(b_):
                """One AllReduce per batch (collectives are latency-bound,
                so fewer+bigger beats chunked)."""
                nc.gpsimd.collective_compute(
                    "AllReduce", OP.add,
                    ins=[cc_in[b_].opt()],
                    outs=[cc_out[b_].opt()],
                    replica_groups=[list(range(NCORES))])

            def emit_a_repack(b_):
                nc.sync.dma_start(ccall[b_][:], cc_out[b_][:])
                nc.sync.dma_start(dtin_sb[b_][:], cc_out[b_][0:DS, :])

            def emit_a_dt_sig(b_, ch, fast=False):
                """dt sigmoid pass for one chunk:
                md = ln(sigmoid(-(dt_raw + b_dt))) = -softplus(.)
                fast=True routes the PSUM through the (then idle) y banks
                for 4-deep rotation so all 8 sigmoids pipeline."""
                t0 = ch * LTA
                for blk2 in range(NBLK):
                    if fast:
                        psd = ps_y.tile([128, LTA], F32,
                                        tag=f"y{(2 * ch + blk2) % 4}",
                                        name=f"psd{b_}{ch}{blk2}")
                    else:
                        psd = ps_small.tile([128, LTA], F32, tag="ps_small")
                    nc.tensor.matmul(
                        psd[:], wdt_sb[:, blk2 * 128:(blk2 + 1) * 128],
                        dtin_sb[b_][:, t0:t0 + LTA],
                        start=True, stop=True)
                    nc.scalar.activation(
                        md[b_][blk2][:, t0:t0 + LTA], psd[:],
                        AF.Sigmoid, bias=bdt_sb[:, blk2, :], scale=-1.0)

            def emit_a_dt_ln(b_, ch):
                t0 = ch * LTA
                for blk2 in range(NBLK):
                    nc.scalar.activation(
                        md[b_][blk2][:, t0:t0 + LTA],
                        md[b_][blk2][:, t0:t0 + LTA], AF.Ln)

            def emit_a_dt(b_, ch):
                emit_a_dt_sig(b_, ch)
                emit_a_dt_ln(b_, ch)

            def alloc_dtx(b_):
                return [pb.tile([128, L], BF16, tag=f"dtx{blk}", bufs=1,
                                name=f"dtx{b_}{blk}") for blk in range(NBLK)]

            def emit_dtx(b_, dtx):
                for blk in range(NBLK):
                    nc.vector.tensor_mul(dtx[blk][:], md[b_][blk][:],
                                         xcv[b_][blk][:])

            def emit_phase_b_n(b_, blk, n, dtx, y_ps):
                """SSM channel n for one channel-block of batch b_."""
                bb = pb.tile([128, L], BF16, tag="bbn", bufs=3,
                             name=f"bb{b_}{blk}{n}")
                nc.sync.dma_start(
                    bb[:],
                    ccall[b_][DS + n:DS + n + 1, :].broadcast_to([128, L]))
                # dA_n = exp(A[:, n] * md)   (md = -dt)
                da = pb.tile([128, L], F32, tag="dan", bufs=3,
                             name=f"da{b_}{blk}{n}")
                nc.scalar.activation(da[:], md[b_][blk][:], AF.Exp,
                                     scale=a_sb[:, blk, n:n + 1])
                # u_n = dtx * B_n  (DVE bf16 2x)
                u = pb.tile([128, L], BF16, tag="un", bufs=3,
                            name=f"u{b_}{blk}{n}")
                nc.vector.tensor_mul(u[:], dtx[blk][:], bb[:])
                # full-length scan
                h = pb.tile([128, L], BF16, tag="hn", bufs=3,
                            name=f"h{b_}{blk}{n}")
                nc.vector.tensor_tensor_scan(h[:], da[:], u[:],
                                             0.0, OP.mult, OP.add)
                # y += h_n on the tensor engine (identity matmul)
                for pt in range(NPT):
                    nc.tensor.matmul(
                        y_ps[pt][:], identb_sb[:],
                        h[:, pt * 512:(pt + 1) * 512],
                        start=(n == 0), stop=False)

            def emit_gate(b_, blk, y_ps):
                yin = pb.tile([128, L], BF16, tag=f"yin{blk}", bufs=1,
                              name=f"yin{b_}{blk}")
                for pt in range(NPT):
                    # y += x_conv * D via diag(D) matmul; stage y through
                    # scalar as bf16 so the gate mul runs at DVE 2x
                    nc.tensor.matmul(
                        y_ps[pt][:], diagd_sb[:, blk, :],
                        xcv[b_][blk][:, pt * 512:(pt + 1) * 512],
                        start=False, stop=True)
                    nc.vector.tensor_mul(
                        yin[:, pt * 512:(pt + 1) * 512], y_ps[pt][:],
                        zac[b_][blk][:, pt * 512:(pt + 1) * 512])
                return yin

            def emit_out_proj_mt(b_, yins, mt):
                # 4-deep PSUM rotation across ps_in(x2)/ps_cv/ps_small
                pools = [(ps_in, "ps_in"), (ps_in, "ps_in"),
                         (ps_cv, "ps_cv"), (ps_small, "ps_small")]
                pso = []
                for i in range(2):
                    pool, tag = pools[(2 * mt + i) % 4]
                    pso.append(pool.tile([128, 512], F32, tag=tag,
                                         name=f"pso{b_}{mt}{i}"))
                for dmh in range(2):
                    for blk in range(NBLK):
                        nc.tensor.matmul(
                            pso[dmh][:],
                            yins[blk][:, mt * 128:(mt + 1) * 128],
                            wout_sb[:, blk, dmh * 512:(dmh + 1) * 512],
                            start=(blk == 0), stop=(blk == NBLK - 1))
                osb = pb.tile([128, DM], F16, tag="osb")
                nc.scalar.copy(osb[:, 0:512], pso[0][:])
                nc.scalar.copy(osb[:, 512:DM], pso[1][:])
                nc.scalar.dma_start(
                    out_d[b_, mt * 128:(mt + 1) * 128, :], osb[:])

            def emit_phase_b_blk(b_, blk, dtx, interleave=None):
                y_ps = [ps_y.tile([128, 512], F32, tag=f"y{pt}", bufs=1,
                                  name=f"yps{b_}{blk}{pt}") for pt in range(NPT)]
                for n in range(16):
                    emit_phase_b_n(b_, blk, n, dtx, y_ps)
                    if interleave and n in interleave:
                        interleave[n]()
                return emit_gate(b_, blk, y_ps)

            # ================= schedule =================
            # Warm-up collective first: absorbs the cross-core rendezvous /
            # launch-skew latency while phase A computes.
            nc.sync.dma_start(warm_in[:], a_d[0:2, :])
            nc.gpsimd.collective_compute(
                "AllReduce", OP.add,
                ins=[warm_in.opt()], outs=[warm_out.opt()],
                replica_groups=[list(range(NCORES))])

            # A(b0) x-branch; single AR; z-chunks fill the PE during the
            # AR's latency; dt immediately after the repack.
            xsq = [load_xs(0, ch) for ch in range(NCHA)]
            wdt_sb, a_sb, bdt_sb, identb_sb, diagd_sb, wout_sb = \
                load_late_weights()
            for ch in range(NCHA):
                emit_a_x(0, ch, xsq[ch])
            emit_a_comm(0)
            emit_a_repack(0)
            for ch in range(NCHA):
                emit_a_dt_sig(0, ch, fast=True)
            for ch in range(NCHA):
                emit_a_dt_ln(0, ch)
            for ch in range(NCHA):
                emit_a_z(0, ch)

            dtx0 = alloc_dtx(0)
            dtx1 = alloc_dtx(1)
            emit_dtx(0, dtx0)

            # A(b1) x-branch spread across B(b0)/blk0; everything else
            # (z, dt, dtx) across blk1.
            xs1 = {}

            def a1x(ch, blk):
                def f():
                    if blk == 0:
                        xs1[ch] = load_xs(1, ch, tag="xs")
                    emit_a_x_half(1, ch, blk, xs1[ch])
                    if ch == NCHA - 1 and blk == NBLK - 1:
                        emit_a_comm(1)
                return f

            def a1z(ch, blk):
                def f():
                    if blk == 0:
                        xs1[ch] = load_xs(1, ch, tag="xs")
                    emit_a_z_half(1, ch, blk, xs1[ch])
                return f

            il0 = {2 * i + 1: a1x(i // 2, i % 2) for i in range(8)}
            il1 = {
                1: lambda: (a1z(0, 0)(), a1z(0, 1)()),
                3: lambda: (emit_a_repack(1), a1z(1, 0)()),
                5: lambda: (a1z(1, 1)(), emit_a_dt(1, 0)),
                7: lambda: (a1z(2, 0)(), emit_a_dt(1, 1)),
                9: lambda: (a1z(2, 1)(), emit_a_dt(1, 2)),
                11: lambda: (a1z(3, 0)(), emit_a_dt(1, 3)),
                13: lambda: (a1z(3, 1)(),),
                15: lambda: emit_dtx(1, dtx1),
            }

            yins0 = {}
            yins1 = {}
            yins0[0] = emit_phase_b_blk(0, 0, dtx0, il0)
            yins0[1] = emit_phase_b_blk(0, 1, dtx0, il1)

            # out_proj(b0) interleaved into B(b1)/blk0 (2 mt per odd slot)
            def op0(mt):
                def f():
                    emit_out_proj_mt(0, yins0, 2 * mt)
                    emit_out_proj_mt(0, yins0, 2 * mt + 1)
                return f

            ilop = {2 * i + 1: op0(i) for i in range(8)}
            yins1[0] = emit_phase_b_blk(1, 0, dtx1, ilop)
            yins1[1] = emit_phase_b_blk(1, 1, dtx1)
            for mt in range(L // 128):
                emit_out_proj_mt(1, yins1, mt)

    nc.compile()
    return nc


_NC_CACHE = {}


def _get_nc():
    if "nc" not in _NC_CACHE:
        _NC_CACHE["nc"] = build_nc()
    return _NC_CACHE["nc"]


def make_in_maps(x, W_in, conv_w, conv_b, W_x, W_dt, b_dt, A_log, D, W_out):
    x = np.asarray(x, np.float32)
    W_in = np.asarray(W_in, np.float32)
    conv_w = np.asarray(conv_w, np.float32)
    conv_b = np.asarray(conv_b, np.float32)
    W_x = np.asarray(W_x, np.float32)
    W_dt = np.asarray(W_dt, np.float32)
    b_dt = np.asarray(b_dt, np.float32)
    A_log = np.asarray(A_log, np.float32)
    D = np.asarray(D, np.float32)
    W_out = np.asarray(W_out, np.float32)

    xt = np.ascontiguousarray(x.transpose(0, 2, 1)).reshape(B, KBLK, 128, L).astype(ml_dtypes.bfloat16)
    A = np.exp(A_log)  # positive |A|; md = -softplus(dt) on device

    in_maps = []
    for c in range(NCORES):
        lo = c * DIL
        sl = slice(lo, lo + DIL)
        # diag conv weights: [NBLK, DC, 128, 128] -> [DIL, DC*128]
        cd = np.zeros((NBLK, DC, 128, 128), np.float32)
        for blk in range(NBLK):
            for k in range(DC):
                np.fill_diagonal(cd[blk, k], conv_w[lo + blk * 128:
                                                    lo + (blk + 1) * 128, k])
        cd = cd.transpose(0, 2, 1, 3).reshape(DIL, DC * 128)
        in_maps.append({
            "x_t": xt,
            "win": np.ascontiguousarray(
                np.concatenate([W_in[:, sl], W_in[:, DI + lo:DI + lo + DIL]],
                               axis=1)).astype(ml_dtypes.bfloat16),
            "wout": np.ascontiguousarray(W_out[sl]).astype(ml_dtypes.bfloat16),
            "wx": np.ascontiguousarray(
                np.concatenate([W_x[sl, :DS], -W_x[sl, DS:]], axis=1)
            ).astype(ml_dtypes.bfloat16),
            "wdt": np.ascontiguousarray(W_dt[:, sl]).astype(ml_dtypes.bfloat16),
            "a": np.ascontiguousarray(A[sl]),
            "convdiag": np.ascontiguousarray(cd).astype(ml_dtypes.bfloat16),
            "convb": np.ascontiguousarray(conv_b[sl, None]),
            "bdt": np.ascontiguousarray(-b_dt[sl, None]),
            "identb": np.eye(128, dtype=ml_dtypes.bfloat16),
            "diagd": np.stack([np.diag(D[lo + k * 128:lo + (k + 1) * 128])
                               for k in range(NBLK)]).reshape(DIL, 128)
                       .astype(ml_dtypes.bfloat16),
        })
    return in_maps


def kernel(**inputs):
    nc = _get_nc()
    in_maps = make_in_maps(**inputs)
    res = run_bass_kernel_spmd(nc, in_maps, list(range(NCORES)))
    out = np.zeros((B, L, DM), np.float32)
    for c in range(NCORES):
        out += res.results[c]["out_p"].astype(np.float32)
    return out
